# revision 1
# baseline (speedup 1.0000x reference)
"""3-layer GAT (DGL-style GATConv) on 8 Trainium2 NeuronCores via Bass/Tile.

Strategy (graph/data parallel, dst-sharded):
- Sort edges by dst on host; shard dst nodes across 8 cores.
- Per layer: each core computes projected features f = h @ W for its node
  shard, all-gathers a bf16 f table, then pulls f[src] per edge with
  dma_gather (256B rows), builds per-128-dst-block one-hot matrices on DVE and
  aggregates alpha-weighted features with PE matmuls accumulating in PSUM.
- Edge softmax: a = exp(leaky_relu(el[src]+er[dst])) computed as
  max(exp(x), exp(0.2x)); z aggregated via the same one-hot matmuls (extra rhs
  columns); divide by z at the dst-node level (no per-edge z gather).
- er[dst] per edge via a second dma_gather from a local [NBLK*128,128]bf16
  table whose rows hold er in cols 0:4.
- Layer 2 (Fout=4*128) is factorized: aggregate alpha-weighted INPUT h per
  head (4*128 cols), then project with W2 per head after transposing.
- Final: relu, per-core max-pool, AllReduce(max), fc + softmax (replicated).
"""
import numpy as np
import ml_dtypes

BF16 = ml_dtypes.bfloat16
P = 128
NC = 8


def _ceil(a, b):
    return -(-a // b)


def _wrap16(seq):
    """dma_gather index layout: [128, n/16] int16, idx i at [i%16, i//16], replicated."""
    n = seq.shape[0]
    assert n % 16 == 0
    w = seq.reshape(n // 16, 16).T.astype(np.int16)  # [16, n/16]
    return np.tile(w, (8, 1))  # [128, n/16]


def _preprocess(src, dst, N, E):
    """Sort by dst, shard, pad blocks to common chunk counts, build gather
    index arrays."""
    SH = N // NC
    NBLK = _ceil(SH, P)
    order = np.argsort(dst, kind="stable")
    ss = src[order].astype(np.int64)
    dd = dst[order].astype(np.int64)
    core_of = dd // SH
    blk_of = (dd % SH) // P

    counts = np.zeros((NC, NBLK), np.int64)
    np.add.at(counts, (core_of, blk_of), 1)
    chunks_b = [max(1, _ceil(int(counts[:, b].max()), P)) for b in range(NBLK)]
    TOT_CH = sum(chunks_b)

    run_off = np.zeros(NC * NBLK + 1, np.int64)
    np.cumsum(counts.reshape(-1), out=run_off[1:])

    groups = [list(range(g * 2, min(g * 2 + 2, NBLK))) for g in range(_ceil(NBLK, 2))]
    grp_ch = [sum(chunks_b[b] for b in g) for g in groups]
    grp_choff = np.zeros(len(groups) + 1, np.int64)
    np.cumsum(grp_ch, out=grp_choff[1:])

    core_arrays = []
    for c in range(NC):
        src_all = np.zeros(TOT_CH * P, np.int64)
        dstm_all = np.full(TOT_CH * P, -1.0, np.float32)
        dstl_all = np.zeros(TOT_CH * P, np.int64)
        off = 0
        for b in range(NBLK):
            r0, r1 = run_off[c * NBLK + b], run_off[c * NBLK + b + 1]
            k = r1 - r0
            kpad = chunks_b[b] * P
            assert k <= kpad, f"block overflow core {c} blk {b}: {k} > {kpad}"
            src_all[off:off + k] = ss[r0:r1]
            dstm_all[off:off + k] = dd[r0:r1] - (c * SH + b * P)
            dstl_all[off:off + k] = dd[r0:r1] - c * SH
            off += kpad
        fidx = np.zeros((P, TOT_CH * 8), np.int16)
        eidx = np.zeros((P, TOT_CH * 8), np.int16)
        for g in range(len(groups)):
            e0, e1 = grp_choff[g] * P, grp_choff[g + 1] * P
            fidx[:, grp_choff[g] * 8:grp_choff[g + 1] * 8] = _wrap16(src_all[e0:e1])
            eidx[:, grp_choff[g] * 8:grp_choff[g + 1] * 8] = _wrap16(dstl_all[e0:e1])
        dstm = np.ascontiguousarray(dstm_all.reshape(TOT_CH, P).T).astype(BF16)
        core_arrays.append({"fidx": fidx, "eidx": eidx, "dstm": dstm})

    sched = {
        "SH": SH, "NBLK": NBLK, "chunks_b": chunks_b, "TOT_CH": TOT_CH,
        "groups": groups, "grp_ch": grp_ch, "grp_choff": grp_choff,
    }
    return sched, core_arrays


def _build_program(sched, FIN, phase=6):
    """Trace the full 3-layer GAT program for one core (SPMD across 8)."""
    import concourse.bacc as bacc
    import concourse.mybir as mybir
    import concourse.tile as tile
    from concourse.masks import make_identity

    dt = mybir.dt
    SH, NBLK, chunks_b = sched["SH"], sched["NBLK"], sched["chunks_b"]
    TOT_CH, groups, grp_choff = sched["TOT_CH"], sched["groups"], sched["grp_choff"]
    N = SH * NC
    NTAB = NBLK * P
    MAXG = max(sched["grp_ch"])
    AF = mybir.ActivationFunctionType
    OP = mybir.AluOpType

    nc = bacc.Bacc("TRN2", target_bir_lowering=False, debug=False, num_devices=NC)

    xT_in = nc.declare_dram_parameter("xT", [FIN, SH], dt.float32, isOutput=False)
    dstm_in = nc.declare_dram_parameter("dstm", [P, TOT_CH], dt.bfloat16, isOutput=False)
    fidx_in = nc.declare_dram_parameter("fidx", [P, TOT_CH * 8], dt.int16, isOutput=False)
    eidx_in = nc.declare_dram_parameter("eidx", [P, TOT_CH * 8], dt.int16, isOutput=False)
    iota_in = nc.declare_dram_parameter("iota", [P, P], dt.bfloat16, isOutput=False)
    W0_in = nc.declare_dram_parameter("W0", [FIN, P], dt.float32, isOutput=False)
    W1_in = nc.declare_dram_parameter("W1", [P, P], dt.float32, isOutput=False)
    W2bf_in = nc.declare_dram_parameter("W2bf", [P, 4 * P], dt.bfloat16, isOutput=False)
    alrep0_in = nc.declare_dram_parameter("alrep0", [P, P], dt.bfloat16, isOutput=False)
    alrep1_in = nc.declare_dram_parameter("alrep1", [P, P], dt.bfloat16, isOutput=False)
    armat0_in = nc.declare_dram_parameter("armat0", [P, 4], dt.float32, isOutput=False)
    armat1_in = nc.declare_dram_parameter("armat1", [P, 4], dt.float32, isOutput=False)
    wal2_in = nc.declare_dram_parameter("wal2", [P, 8], dt.float32, isOutput=False)
    fcw_in = nc.declare_dram_parameter("fcw", [P, 4 * 8], dt.float32, isOutput=False)
    fcb_in = nc.declare_dram_parameter("fcb", [1, 8], dt.float32, isOutput=False)
    out_ext = nc.declare_dram_parameter("out", [1, 8], dt.float32, isOutput=True)
    dbg_ext = nc.declare_dram_parameter("dbg", [P, 512], dt.float32, isOutput=True)

    def dram(name, shape, dtype, shared=False):
        return nc.dram_tensor(name, shape, dtype,
                              addr_space="Shared" if shared else "Local")

    fsh = [dram(f"fsh{l}", [NTAB, 256 if l == 2 else P], dt.bfloat16) for l in range(3)]
    ftab = [dram(f"ftab{l}", [N, 256 if l == 2 else P], dt.bfloat16, shared=True)
            for l in range(3)]
    ertab = [dram(f"ertab{l}", [NTAB, P], dt.bfloat16) for l in range(3)]
    pmax_in = dram("pmax_in", [P, 4], dt.float32)
    pmax_out = dram("pmax_out", [P, 4], dt.float32, shared=True)
    rg = [list(range(NC))]

    with tile.TileContext(nc) as tc:
        with (
            tc.tile_pool(name="const", bufs=1) as cp,
            tc.tile_pool(name="pers", bufs=1) as pers,
            tc.tile_pool(name="gath", bufs=2) as gp,
            tc.tile_pool(name="wk", bufs=3) as wk,
            tc.tile_pool(name="ep", bufs=2) as ep,
            tc.tile_pool(name="psum", bufs=6, space="PSUM") as pp,
        ):
            f32, bf16 = dt.float32, dt.bfloat16

            def load_const(name, src_ap, shape, dtype):
                t = cp.tile(shape, dtype, tag=name)
                nc.sync.dma_start(out=t[:], in_=src_ap)
                return t

            iota_sb = load_const("iota", iota_in[:], [P, P], bf16)
            dstm_sb = load_const("dstm", dstm_in[:], [P, TOT_CH], bf16)
            fidx_sb = load_const("fidx", fidx_in[:], [P, TOT_CH * 8], dt.int16)
            eidx_sb = load_const("eidx", eidx_in[:], [P, TOT_CH * 8], dt.int16)
            W0_sb = load_const("W0", W0_in[:], [FIN, P], f32)
            W1_sb = load_const("W1", W1_in[:], [P, P], f32)
            W2bf_sb = load_const("W2bf", W2bf_in[:], [P, 4 * P], bf16)
            alrep_sb = [load_const("alrep0", alrep0_in[:], [P, P], bf16),
                        load_const("alrep1", alrep1_in[:], [P, P], bf16), None]
            armat_sb = [load_const("armat0", armat0_in[:], [P, 4], f32),
                        load_const("armat1", armat1_in[:], [P, 4], f32), None]
            wal2_sb = load_const("wal2", wal2_in[:], [P, 8], f32)
            fcw_sb = load_const("fcw", fcw_in[:], [P, 4 * 8], f32)
            fcb_sb = load_const("fcb", fcb_in[:], [1, 8], f32)
            ident = cp.tile([P, P], f32, tag="identf")
            make_identity(nc, ident[:])
            identbf = cp.tile([P, P], bf16, tag="identbf")
            nc.vector.tensor_copy(out=identbf[:], in_=ident[:])

            def stage_prep(l, hT_sb):
                """Build the gather tables for layer l from hT (f32 [128, SH])
                and AllGather. l<2: table rows = f = h@W (projected, bf16) and
                er table; l==2: rows = [h | el2 | pad] plus er2 table."""
                W_sb = [W0_sb, W1_sb, None][l]
                row_w = 132 if l == 2 else P
                fbf = pers.tile([P, NBLK * row_w], bf16, tag="fbf")
                ersb = pers.tile([P, NBLK * 4], bf16, tag="ersb")
                if l < 2:
                    fT = pers.tile([P, SH], f32, tag="fT")
                    nn = 0
                    while nn < SH:
                        w = min(512, SH - nn)
                        ftp = pp.tile([P, 512], f32, tag="pp")
                        nc.tensor.matmul(out=ftp[:, :w], lhsT=W_sb[:],
                                         rhs=hT_sb[:, nn:nn + w],
                                         start=True, stop=True)
                        nc.vector.tensor_copy(out=fT[:, nn:nn + w], in_=ftp[:, :w])
                        nn += w
                    srcT = fT
                else:
                    srcT = hT_sb
                for b in range(NBLK):
                    nv = min(P, SH - b * P)
                    co = b * row_w
                    erp = pp.tile([P, 8], f32, tag="pp")
                    rhs_er = wal2_sb[:] if l == 2 else armat_sb[l][:]
                    ncols = 8 if l == 2 else 4
                    nc.tensor.matmul(out=erp[:nv, :ncols],
                                     lhsT=srcT[:, b * P:b * P + nv],
                                     rhs=rhs_er, start=True, stop=True)
                    if l == 2:
                        nc.vector.tensor_copy(out=fbf[:nv, co + 128:co + 132],
                                              in_=erp[:nv, 0:4])
                        nc.vector.tensor_copy(out=ersb[:nv, b * 4:(b + 1) * 4],
                                              in_=erp[:nv, 4:8])
                    else:
                        nc.vector.tensor_copy(out=ersb[:nv, b * 4:(b + 1) * 4],
                                              in_=erp[:nv, 0:4])
                    trp = pp.tile([P, P], f32, tag="pp")
                    nc.tensor.transpose(out=trp[:nv, :],
                                        in_=srcT[:, b * P:b * P + nv],
                                        identity=ident[:])
                    nc.vector.tensor_copy(out=fbf[:nv, co:co + P], in_=trp[:nv, :])
                nc.sync.dma_start(
                    out=fsh[l].rearrange("(b p) f -> p b f", p=P)[:, :, 0:row_w],
                    in_=fbf[:].rearrange("p (b w) -> p b w", w=row_w))
                nc.sync.dma_start(
                    out=ertab[l].rearrange("(b p) f -> p b f", p=P)[:, :, 0:4],
                    in_=ersb[:].rearrange("p (b w) -> p b w", w=4))
                nc.gpsimd.collective_compute(
                    "AllGather", OP.bypass,
                    ins=[fsh[l][:SH, :]], outs=[ftab[l][:]], replica_groups=rg)

            def _epilogue01(b, blk_ps, hT_next):
                nv = min(P, SH - b * P)
                zi = ep.tile([P, 4], f32, tag="zi")
                nc.vector.tensor_scalar(out=zi[:], in0=blk_ps[:, 128:132],
                                        scalar1=1e-30, scalar2=None, op0=OP.add)
                nc.vector.reciprocal(out=zi[:], in_=zi[:])
                hdiv = ep.tile([P, P], f32, tag="hdiv")
                nc.vector.tensor_tensor(
                    out=hdiv[:].rearrange("p (h c) -> p h c", c=32),
                    in0=blk_ps[:, 0:128].rearrange("p (h c) -> p h c", c=32),
                    in1=zi[:].unsqueeze(-1).to_broadcast([P, 4, 32]),
                    op=OP.mult)
                hre = ep.tile([P, P], f32, tag="hre")
                nc.scalar.activation(out=hre[:], in_=hdiv[:], func=AF.Relu)
                trp = pp.tile([P, P], f32, tag="pp")
                nc.tensor.transpose(out=trp[:], in_=hre[:], identity=ident[:])
                nc.vector.tensor_copy(out=hT_next[:, b * P:b * P + nv],
                                      in_=trp[:, :nv])

            def _epilogue2(b, psA, psB, acc_max):
                nv = min(P, SH - b * P)
                zi = ep.tile([P, 4], f32, tag="zi")
                nc.vector.tensor_scalar(out=zi[:], in0=psB[:, 256:260],
                                        scalar1=1e-30, scalar2=None, op0=OP.add)
                nc.vector.reciprocal(out=zi[:], in_=zi[:])
                agg = ep.tile([P, 512], bf16, tag="agg")
                nc.vector.tensor_copy(out=agg[:, 0:256], in_=psA[:, 0:256])
                nc.vector.tensor_copy(out=agg[:, 256:512], in_=psB[:, 0:256])
                o2 = pp.tile([P, 512], f32, tag="pp")
                for h in range(4):
                    trp = pp.tile([P, P], bf16, tag="pp")
                    nc.tensor.transpose(out=trp[:], in_=agg[:, h * P:(h + 1) * P],
                                        identity=identbf[:])
                    aggT = ep.tile([P, P], bf16, tag="aggT")
                    nc.vector.tensor_copy(out=aggT[:], in_=trp[:])
                    nc.tensor.matmul(out=o2[:, h * P:(h + 1) * P], lhsT=aggT[:],
                                     rhs=W2bf_sb[:, h * P:(h + 1) * P],
                                     start=True, stop=True)
                o2r = ep.tile([P, 512], f32, tag="o2r")
                for h in range(4):
                    nc.scalar.activation(out=o2r[:, h * P:(h + 1) * P],
                                         in_=o2[:, h * P:(h + 1) * P],
                                         func=AF.Relu, scale=zi[:, h:h + 1])
                nc.vector.tensor_tensor(out=acc_max[:nv, :], in0=acc_max[:nv, :],
                                        in1=o2r[:nv, :], op=OP.max)

            def layer_main(l, hT_next, acc_max):
                elem = 256 if l == 2 else P
                aw = 512 if l == 2 else 128  # alpha col offset within 516-wide row
                ch0 = 0
                cur_b = 0
                done_in_b = 0
                blk_ps = blk_psB = None
                for g, blks in enumerate(groups):
                    gch = sched["grp_ch"][g]
                    o8 = int(grp_choff[g]) * 8
                    n_idx = gch * P
                    fg = gp.tile([P, MAXG, elem], bf16, tag="fg")
                    nc.gpsimd.dma_gather(
                        out_ap=fg[:, :gch, :], in_ap=ftab[l][:],
                        idxs_ap=fidx_sb[:, o8:o8 + gch * 8],
                        num_idxs=n_idx, num_idxs_reg=n_idx, elem_size=elem,
                        single_packet=False)
                    eg = gp.tile([P, MAXG, P], bf16, tag="eg")
                    nc.gpsimd.dma_gather(
                        out_ap=eg[:, :gch, :], in_ap=ertab[l][:],
                        idxs_ap=eidx_sb[:, o8:o8 + gch * 8],
                        num_idxs=n_idx, num_idxs_reg=n_idx, elem_size=P,
                        single_packet=False)
                    for s0 in range(0, gch, 4):
                        sub = min(4, gch - s0)
                        ch = ch0 + s0
                        gt = wk.tile([P, 4 * 516], bf16, tag="gt")
                        gtv = gt[:].rearrange("p (s w) -> p s w", w=516)
                        oh = wk.tile([P, 4, P], bf16, tag="oh")
                        nc.vector.tensor_tensor(
                            out=oh[:, :sub, :],
                            in0=dstm_sb[:, ch:ch + sub].unsqueeze(-1).to_broadcast([P, sub, P]),
                            in1=iota_sb[:].unsqueeze(1).to_broadcast([P, sub, P]),
                            op=OP.is_equal)
                        e4 = wk.tile([P, 16], f32, tag="e4")
                        e4v = e4[:, :sub * 4].rearrange("p (s h) -> p s h", h=4)
                        if l < 2:
                            tmp = wk.tile([P, 4 * P], bf16, tag="tmp")
                            nc.vector.tensor_tensor(
                                out=tmp[:, :sub * P].rearrange("p (s f) -> p s f", f=P),
                                in0=fg[:, s0:s0 + sub, :],
                                in1=alrep_sb[l][:].unsqueeze(1).to_broadcast([P, sub, P]),
                                op=OP.mult)
                            el4 = wk.tile([P, 16], f32, tag="el4")
                            nc.vector.reduce_sum(
                                out=el4[:, :sub * 4],
                                in_=tmp[:, :sub * P].rearrange("p (q c) -> p q c", c=32),
                                axis=mybir.AxisListType.X)
                            nc.vector.tensor_tensor(
                                out=e4v,
                                in0=el4[:, :sub * 4].rearrange("p (s h) -> p s h", h=4),
                                in1=eg[:, s0:s0 + sub, 0:4],
                                op=OP.add)
                        else:
                            nc.vector.tensor_tensor(
                                out=e4v,
                                in0=fg[:, s0:s0 + sub, 128:132],
                                in1=eg[:, s0:s0 + sub, 0:4],
                                op=OP.add)
                        a1 = wk.tile([P, 16], bf16, tag="a1")
                        a2 = wk.tile([P, 16], bf16, tag="a2")
                        nc.scalar.activation(out=a1[:, :sub * 4], in_=e4[:, :sub * 4],
                                             func=AF.Exp)
                        nc.scalar.activation(out=a2[:, :sub * 4], in_=e4[:, :sub * 4],
                                             func=AF.Exp, scale=0.2)
                        nc.vector.tensor_tensor(
                            out=gtv[:, :sub, aw:aw + 4],
                            in0=a1[:, :sub * 4].rearrange("p (s h) -> p s h", h=4),
                            in1=a2[:, :sub * 4].rearrange("p (s h) -> p s h", h=4),
                            op=OP.max)
                        if l < 2:
                            nc.vector.tensor_tensor(
                                out=gtv[:, :sub, 0:P].rearrange("p s (h c) -> p s h c", c=32),
                                in0=fg[:, s0:s0 + sub, :].rearrange("p s (h c) -> p s h c", c=32),
                                in1=gtv[:, :sub, aw:aw + 4].unsqueeze(-1).to_broadcast([P, sub, 4, 32]),
                                op=OP.mult)
                        else:
                            for c in range(sub):
                                nc.vector.tensor_tensor(
                                    out=gtv[:, c, 0:512].rearrange("p (h f) -> p h f", f=P),
                                    in0=fg[:, s0 + c, 0:P].unsqueeze(1).to_broadcast([P, 4, P]),
                                    in1=gtv[:, c, aw:aw + 4].unsqueeze(-1).to_broadcast([P, 4, P]),
                                    op=OP.mult)
                        for c in range(sub):
                            if done_in_b == 0:
                                blk_ps = pp.tile([P, 512], f32, tag="pp")
                                if l == 2:
                                    blk_psB = pp.tile([P, 512], f32, tag="pp")
                            first = done_in_b == 0
                            last = done_in_b == chunks_b[cur_b] - 1
                            if l < 2:
                                nc.tensor.matmul(
                                    out=blk_ps[:, 0:132], lhsT=oh[:, c, :],
                                    rhs=gt[:, c * 516:c * 516 + 132],
                                    start=first, stop=last)
                            else:
                                nc.tensor.matmul(
                                    out=blk_ps[:, 0:256], lhsT=oh[:, c, :],
                                    rhs=gt[:, c * 516:c * 516 + 256],
                                    start=first, stop=last)
                                nc.tensor.matmul(
                                    out=blk_psB[:, 0:260], lhsT=oh[:, c, :],
                                    rhs=gt[:, c * 516 + 256:c * 516 + 516],
                                    start=first, stop=last)
                            done_in_b += 1
                            if last:
                                if l < 2:
                                    _epilogue01(cur_b, blk_ps, hT_next)
                                else:
                                    _epilogue2(cur_b, blk_ps, blk_psB, acc_max)
                                done_in_b = 0
                                cur_b += 1
                    ch0 += gch

            # ================= run the network =================
            def dump_dbg(ap_f32_cols):
                """ap_f32_cols: list of (sbuf_ap, col0, width)"""
                dt_ = pers.tile([P, 512], f32, tag="dbgt")
                nc.gpsimd.memset(dt_[:], 0.0)
                for ap, c0, w in ap_f32_cols:
                    nc.vector.tensor_copy(out=dt_[:, c0:c0 + w], in_=ap)
                nc.sync.dma_start(out=dbg_ext[:], in_=dt_[:])
                dd = ep.tile([1, 8], f32, tag="ot")
                nc.gpsimd.memset(dd[:], 0.5)
                nc.sync.dma_start(out=out_ext[:], in_=dd[:])

            def dump_tab(l):
                w = 256 if l == 2 else P
                tb = ep.tile([P, w], bf16, tag="dump1")
                nc.sync.dma_start(out=tb[:], in_=ftab[l][0:P, :])
                er_ = ep.tile([P, 4], bf16, tag="dump2")
                nc.sync.dma_start(out=er_[:], in_=ertab[l][0:P, 0:4])
                cols = [(tb[:, :P], 0, P), (er_[:], 128, 4)]
                if l == 2:
                    cols.append((tb[:, 128:132], 140, 4))
                dump_dbg(cols)

            hT0 = pers.tile([P, SH], f32, tag="hT0")
            nc.sync.dma_start(out=hT0[:FIN, :], in_=xT_in[:])
            stage_prep(0, hT0)
            if phase == 0:
                dump_tab(0)
            if phase >= 1:
                hT1 = pers.tile([P, SH], f32, tag="hT1")
                layer_main(0, hT1, None)
                if phase == 1:
                    dump_dbg([(hT1[:, 0:min(512, SH)], 0, min(512, SH))])
            if phase >= 2:
                stage_prep(1, hT1)
                if phase == 2:
                    dump_tab(1)
            if phase >= 3:
                hT2 = pers.tile([P, SH], f32, tag="hT2")
                layer_main(1, hT2, None)
                if phase == 3:
                    dump_dbg([(hT2[:, 0:min(512, SH)], 0, min(512, SH))])
            if phase >= 4:
                stage_prep(2, hT2)
                if phase == 4:
                    dump_tab(2)
            if phase >= 5:
                acc_max = pers.tile([P, 512], f32, tag="accmax")
                nc.gpsimd.memset(acc_max[:], 0.0)
                layer_main(2, None, acc_max)
                if phase == 5:
                    dump_dbg([(acc_max[:], 0, 512)])
            def head():

                pooledT = ep.tile([P, 4], f32, tag="pooledT")
                for j in range(4):
                    trp = pp.tile([P, P], f32, tag="pp")
                    nc.tensor.transpose(out=trp[:], in_=acc_max[:, j * P:(j + 1) * P],
                                        identity=ident[:])
                    nc.vector.reduce_max(out=pooledT[:, j:j + 1], in_=trp[:],
                                         axis=mybir.AxisListType.X)
                nc.sync.dma_start(out=pmax_in[:], in_=pooledT[:])
                nc.gpsimd.collective_compute(
                    "AllReduce", OP.max,
                    ins=[pmax_in[:]], outs=[pmax_out[:]], replica_groups=rg)
                pm = ep.tile([P, 4], f32, tag="pm")
                nc.sync.dma_start(out=pm[:], in_=pmax_out[:])
                fcp = pp.tile([1, 8], f32, tag="pp")
                for j in range(4):
                    nc.tensor.matmul(out=fcp[:], lhsT=pm[:, j:j + 1],
                                     rhs=fcw_sb[:, j * 8:(j + 1) * 8],
                                     start=(j == 0), stop=(j == 3))
                lg = ep.tile([1, 8], f32, tag="lg")
                nc.vector.tensor_tensor(out=lg[:], in0=fcp[:], in1=fcb_sb[:], op=OP.add)
                mx = ep.tile([1, 1], f32, tag="mx")
                nc.vector.reduce_max(out=mx[:], in_=lg[:], axis=mybir.AxisListType.X)
                nc.vector.tensor_tensor(out=lg[:], in0=lg[:],
                                        in1=mx[:].to_broadcast([1, 8]), op=OP.subtract)
                ex = ep.tile([1, 8], f32, tag="ex")
                nc.scalar.activation(out=ex[:], in_=lg[:], func=AF.Exp)
                sm = ep.tile([1, 1], f32, tag="sm")
                nc.vector.reduce_sum(out=sm[:], in_=ex[:], axis=mybir.AxisListType.X)
                nc.vector.reciprocal(out=sm[:], in_=sm[:])
                ot = ep.tile([1, 8], f32, tag="ot")
                nc.vector.tensor_tensor(out=ot[:], in0=ex[:],
                                        in1=sm[:].to_broadcast([1, 8]), op=OP.mult)
                nc.sync.dma_start(out=out_ext[:], in_=ot[:])

            # ---- head ----
            if phase >= 6:
                head()

    nc.finalize()
    return nc


def _host_consts(W0, al0, ar0, W1, al1, ar1, W2, al2, ar2, fc_w, fc_b):
    def alflat(al):
        v = al.reshape(-1).astype(np.float32)
        return np.tile(v[None, :], (P, 1)).astype(BF16)

    def armat(ar):
        hh, cc = ar.shape
        m = np.zeros((hh * cc, hh), np.float32)
        for h in range(hh):
            m[h * cc:(h + 1) * cc, h] = ar[h]
        return m

    wal2 = np.zeros((P, 8), np.float32)
    wal2[:, 0:4] = (W2.astype(np.float64) @ armat(al2).astype(np.float64)).astype(np.float32)
    wal2[:, 4:8] = (W2.astype(np.float64) @ armat(ar2).astype(np.float64)).astype(np.float32)
    fcw = np.ascontiguousarray(
        fc_w.reshape(4, P, 8).transpose(1, 0, 2).reshape(P, 32)).astype(np.float32)
    return {
        "W0": np.ascontiguousarray(W0).astype(np.float32),
        "W1": np.ascontiguousarray(W1).astype(np.float32),
        "W2bf": np.ascontiguousarray(W2).astype(BF16),
        "alrep0": alflat(al0), "alrep1": alflat(al1),
        "armat0": armat(ar0), "armat1": armat(ar1),
        "wal2": wal2,
        "fcw": fcw, "fcb": fc_b.reshape(1, 8).astype(np.float32),
        "iota": np.tile(np.arange(P, dtype=np.float32)[None, :], (P, 1)).astype(BF16),
    }


_PROG_CACHE = {}


def run_gat(inputs, src, dst, W0, al0, ar0, W1, al1, ar1, W2, al2, ar2, fc_w, fc_b,
            trace=False):
    from concourse.bass_utils import run_bass_kernel_spmd
    inputs = np.asarray(inputs)
    N, FIN = inputs.shape
    E = np.asarray(src).shape[0]
    sched, core_arrays = _preprocess(np.asarray(src), np.asarray(dst), N, E)
    import os
    phase = int(os.environ.get("GAT_PHASE", "6"))
    key = (N, E, FIN, tuple(sched["chunks_b"]), phase)
    if key not in _PROG_CACHE:
        _PROG_CACHE[key] = _build_program(sched, FIN, phase)
    nc = _PROG_CACHE[key]
    consts = _host_consts(np.asarray(W0), np.asarray(al0), np.asarray(ar0),
                          np.asarray(W1), np.asarray(al1), np.asarray(ar1),
                          np.asarray(W2), np.asarray(al2), np.asarray(ar2),
                          np.asarray(fc_w), np.asarray(fc_b))
    SH = sched["SH"]
    in_maps = []
    for c in range(NC):
        m = dict(consts)
        m.update(core_arrays[c])
        m["xT"] = np.ascontiguousarray(
            inputs[c * SH:(c + 1) * SH, :].T).astype(np.float32)
        in_maps.append(m)
    res = run_bass_kernel_spmd(nc, in_maps, list(range(NC)), trace=trace)
    out = np.asarray(res.results[0]["out"])
    run_gat.last_dbg = np.asarray(res.results[0].get("dbg")) if "dbg" in res.results[0] else None
    return out, res


def kernel(**inputs):
    out, _ = run_gat(**inputs)
    return out



# revision 3
# speedup vs baseline: 1.3154x; 1.3154x over previous
"""3-layer GAT (DGL-style GATConv) on 8 Trainium2 NeuronCores via Bass/Tile.

Strategy (graph/data parallel, dst-sharded):
- Sort edges by dst on host; shard dst nodes across 8 cores.
- Per layer: each core computes projected features f = h @ W for its node
  shard, all-gathers a bf16 f table, then pulls f[src] per edge with
  dma_gather (256B rows), builds per-128-dst-block one-hot matrices on DVE and
  aggregates alpha-weighted features with PE matmuls accumulating in PSUM.
- Edge softmax: a = exp(leaky_relu(el[src]+er[dst])) computed as
  max(exp(x), exp(0.2x)); z aggregated via the same one-hot matmuls (extra rhs
  columns); divide by z at the dst-node level (no per-edge z gather).
- er[dst] per edge via a second dma_gather from a local [NBLK*128,128]bf16
  table whose rows hold er in cols 0:4.
- Layer 2 (Fout=4*128) is factorized: aggregate alpha-weighted INPUT h per
  head (4*128 cols), then project with W2 per head after transposing.
- Final: relu, per-core max-pool, AllReduce(max), fc + softmax (replicated).
"""
import numpy as np
import ml_dtypes

BF16 = ml_dtypes.bfloat16
P = 128
NC = 8


def _ceil(a, b):
    return -(-a // b)


def _wrap16(seq):
    """dma_gather index layout: [128, n/16] int16, idx i at [i%16, i//16], replicated."""
    n = seq.shape[0]
    assert n % 16 == 0
    w = seq.reshape(n // 16, 16).T.astype(np.int16)  # [16, n/16]
    return np.tile(w, (8, 1))  # [128, n/16]


def _preprocess(src, dst, N, E):
    """Sort by dst, shard, pad blocks to common chunk counts, build gather
    index arrays."""
    SH = N // NC
    NBLK = _ceil(SH, P)
    order = np.argsort(dst, kind="stable")
    ss = src[order].astype(np.int64)
    dd = dst[order].astype(np.int64)
    core_of = dd // SH
    blk_of = (dd % SH) // P

    counts = np.zeros((NC, NBLK), np.int64)
    np.add.at(counts, (core_of, blk_of), 1)
    chunks_b = [max(1, _ceil(int(counts[:, b].max()), P)) for b in range(NBLK)]
    TOT_CH = sum(chunks_b)

    run_off = np.zeros(NC * NBLK + 1, np.int64)
    np.cumsum(counts.reshape(-1), out=run_off[1:])

    groups = [list(range(g * 2, min(g * 2 + 2, NBLK))) for g in range(_ceil(NBLK, 2))]
    grp_ch = [sum(chunks_b[b] for b in g) for g in groups]
    grp_choff = np.zeros(len(groups) + 1, np.int64)
    np.cumsum(grp_ch, out=grp_choff[1:])

    core_arrays = []
    for c in range(NC):
        src_all = np.zeros(TOT_CH * P, np.int64)
        dstm_all = np.full(TOT_CH * P, -1.0, np.float32)
        dstl_all = np.zeros(TOT_CH * P, np.int64)
        off = 0
        for b in range(NBLK):
            r0, r1 = run_off[c * NBLK + b], run_off[c * NBLK + b + 1]
            k = r1 - r0
            kpad = chunks_b[b] * P
            assert k <= kpad, f"block overflow core {c} blk {b}: {k} > {kpad}"
            src_all[off:off + k] = ss[r0:r1]
            dstm_all[off:off + k] = dd[r0:r1] - (c * SH + b * P)
            dstl_all[off:off + k] = dd[r0:r1] - c * SH
            off += kpad
        fidx = np.zeros((P, TOT_CH * 8), np.int16)
        eidx = np.zeros((P, TOT_CH * 8), np.int16)
        for g in range(len(groups)):
            e0, e1 = grp_choff[g] * P, grp_choff[g + 1] * P
            fidx[:, grp_choff[g] * 8:grp_choff[g + 1] * 8] = _wrap16(src_all[e0:e1])
            eidx[:, grp_choff[g] * 8:grp_choff[g + 1] * 8] = _wrap16(dstl_all[e0:e1])
        dstm = np.ascontiguousarray(dstm_all.reshape(TOT_CH, P).T).astype(BF16)
        core_arrays.append({"fidx": fidx, "eidx": eidx, "dstm": dstm})

    sched = {
        "SH": SH, "NBLK": NBLK, "chunks_b": chunks_b, "TOT_CH": TOT_CH,
        "groups": groups, "grp_ch": grp_ch, "grp_choff": grp_choff,
    }
    return sched, core_arrays


def _build_program(sched, FIN, phase=6):
    """Trace the full 3-layer GAT program for one core (SPMD across 8)."""
    import concourse.bacc as bacc
    import concourse.mybir as mybir
    import concourse.tile as tile
    from concourse.masks import make_identity

    dt = mybir.dt
    SH, NBLK, chunks_b = sched["SH"], sched["NBLK"], sched["chunks_b"]
    TOT_CH, groups, grp_choff = sched["TOT_CH"], sched["groups"], sched["grp_choff"]
    N = SH * NC
    NTAB = NBLK * P
    MAXG = max(sched["grp_ch"])
    AF = mybir.ActivationFunctionType
    OP = mybir.AluOpType

    nc = bacc.Bacc("TRN2", target_bir_lowering=False, debug=False, num_devices=NC,
                   num_swdge_queues=4)

    xT_in = nc.declare_dram_parameter("xT", [FIN, SH], dt.float32, isOutput=False)
    dstm_in = nc.declare_dram_parameter("dstm", [P, TOT_CH], dt.bfloat16, isOutput=False)
    fidx_in = nc.declare_dram_parameter("fidx", [P, TOT_CH * 8], dt.int16, isOutput=False)
    eidx_in = nc.declare_dram_parameter("eidx", [P, TOT_CH * 8], dt.int16, isOutput=False)
    iota_in = nc.declare_dram_parameter("iota", [P, P], dt.bfloat16, isOutput=False)
    W0_in = nc.declare_dram_parameter("W0", [FIN, P], dt.float32, isOutput=False)
    W1_in = nc.declare_dram_parameter("W1", [P, P], dt.float32, isOutput=False)
    W2bf_in = nc.declare_dram_parameter("W2bf", [P, 4 * P], dt.bfloat16, isOutput=False)
    alrep0_in = nc.declare_dram_parameter("alrep0", [P, P], dt.bfloat16, isOutput=False)
    alrep1_in = nc.declare_dram_parameter("alrep1", [P, P], dt.bfloat16, isOutput=False)
    armat0_in = nc.declare_dram_parameter("armat0", [P, 4], dt.float32, isOutput=False)
    armat1_in = nc.declare_dram_parameter("armat1", [P, 4], dt.float32, isOutput=False)
    wal2_in = nc.declare_dram_parameter("wal2", [P, 8], dt.float32, isOutput=False)
    fcw_in = nc.declare_dram_parameter("fcw", [P, 4 * 8], dt.float32, isOutput=False)
    fcb_in = nc.declare_dram_parameter("fcb", [1, 8], dt.float32, isOutput=False)
    out_ext = nc.declare_dram_parameter("out", [1, 8], dt.float32, isOutput=True)
    dbg_ext = nc.declare_dram_parameter("dbg", [P, 512], dt.float32, isOutput=True)

    def dram(name, shape, dtype, shared=False):
        return nc.dram_tensor(name, shape, dtype,
                              addr_space="Shared" if shared else "Local")

    fsh = [dram(f"fsh{l}", [NTAB, 256 if l == 2 else P], dt.bfloat16) for l in range(3)]
    ftab = [dram(f"ftab{l}", [N, 256 if l == 2 else P], dt.bfloat16, shared=True)
            for l in range(3)]
    ertab = [dram(f"ertab{l}", [NTAB, P], dt.bfloat16) for l in range(3)]
    pmax_in = dram("pmax_in", [P, 4], dt.float32)
    pmax_out = dram("pmax_out", [P, 4], dt.float32, shared=True)
    rg = [list(range(NC))]

    with tile.TileContext(nc) as tc:
        with (
            tc.tile_pool(name="const", bufs=1) as cp,
            tc.tile_pool(name="pers", bufs=1) as pers,
            tc.tile_pool(name="gath", bufs=2) as gp,
            tc.tile_pool(name="wk", bufs=3) as wk,
            tc.tile_pool(name="ep", bufs=2) as ep,
            tc.tile_pool(name="psum", bufs=6, space="PSUM") as pp,
        ):
            f32, bf16 = dt.float32, dt.bfloat16

            def load_const(name, src_ap, shape, dtype):
                t = cp.tile(shape, dtype, tag=name)
                nc.sync.dma_start(out=t[:], in_=src_ap)
                return t

            iota_sb = load_const("iota", iota_in[:], [P, P], bf16)
            dstm_sb = load_const("dstm", dstm_in[:], [P, TOT_CH], bf16)
            fidx_sb = load_const("fidx", fidx_in[:], [P, TOT_CH * 8], dt.int16)
            eidx_sb = load_const("eidx", eidx_in[:], [P, TOT_CH * 8], dt.int16)
            W0_sb = load_const("W0", W0_in[:], [FIN, P], f32)
            W1_sb = load_const("W1", W1_in[:], [P, P], f32)
            W2bf_sb = load_const("W2bf", W2bf_in[:], [P, 4 * P], bf16)
            alrep_sb = [load_const("alrep0", alrep0_in[:], [P, P], bf16),
                        load_const("alrep1", alrep1_in[:], [P, P], bf16), None]
            armat_sb = [load_const("armat0", armat0_in[:], [P, 4], f32),
                        load_const("armat1", armat1_in[:], [P, 4], f32), None]
            wal2_sb = load_const("wal2", wal2_in[:], [P, 8], f32)
            fcw_sb = load_const("fcw", fcw_in[:], [P, 4 * 8], f32)
            fcb_sb = load_const("fcb", fcb_in[:], [1, 8], f32)
            ident = cp.tile([P, P], f32, tag="identf")
            make_identity(nc, ident[:])
            identbf = cp.tile([P, P], bf16, tag="identbf")
            nc.vector.tensor_copy(out=identbf[:], in_=ident[:])

            def stage_prep(l, hT_sb):
                """Build the gather tables for layer l from hT (f32 [128, SH])
                and AllGather. l<2: table rows = f = h@W (projected, bf16) and
                er table; l==2: rows = [h | el2 | pad] plus er2 table."""
                W_sb = [W0_sb, W1_sb, None][l]
                row_w = 132 if l == 2 else P
                fbf = pers.tile([P, NBLK * row_w], bf16, tag="fbf")
                ersb = pers.tile([P, NBLK * 4], bf16, tag="ersb")
                if l < 2:
                    fT = pers.tile([P, SH], f32, tag="fT")
                    nn = 0
                    while nn < SH:
                        w = min(512, SH - nn)
                        ftp = pp.tile([P, 512], f32, tag="pp")
                        nc.tensor.matmul(out=ftp[:, :w], lhsT=W_sb[:],
                                         rhs=hT_sb[:, nn:nn + w],
                                         start=True, stop=True)
                        nc.vector.tensor_copy(out=fT[:, nn:nn + w], in_=ftp[:, :w])
                        nn += w
                    srcT = fT
                else:
                    srcT = hT_sb
                for b in range(NBLK):
                    nv = min(P, SH - b * P)
                    co = b * row_w
                    erp = pp.tile([P, 8], f32, tag="pp")
                    rhs_er = wal2_sb[:] if l == 2 else armat_sb[l][:]
                    ncols = 8 if l == 2 else 4
                    nc.tensor.matmul(out=erp[:nv, :ncols],
                                     lhsT=srcT[:, b * P:b * P + nv],
                                     rhs=rhs_er, start=True, stop=True)
                    if l == 2:
                        nc.vector.tensor_copy(out=fbf[:nv, co + 128:co + 132],
                                              in_=erp[:nv, 0:4])
                        nc.vector.tensor_copy(out=ersb[:nv, b * 4:(b + 1) * 4],
                                              in_=erp[:nv, 4:8])
                    else:
                        nc.vector.tensor_copy(out=ersb[:nv, b * 4:(b + 1) * 4],
                                              in_=erp[:nv, 0:4])
                    trp = pp.tile([P, P], f32, tag="pp")
                    nc.tensor.transpose(out=trp[:nv, :],
                                        in_=srcT[:, b * P:b * P + nv],
                                        identity=ident[:])
                    nc.vector.tensor_copy(out=fbf[:nv, co:co + P], in_=trp[:nv, :])
                nc.sync.dma_start(
                    out=fsh[l].rearrange("(b p) f -> p b f", p=P)[:, :, 0:row_w],
                    in_=fbf[:].rearrange("p (b w) -> p b w", w=row_w))
                nc.sync.dma_start(
                    out=ertab[l].rearrange("(b p) f -> p b f", p=P)[:, :, 0:4],
                    in_=ersb[:].rearrange("p (b w) -> p b w", w=4))
                nc.gpsimd.collective_compute(
                    "AllGather", OP.bypass,
                    ins=[fsh[l][:SH, :]], outs=[ftab[l][:]], replica_groups=rg)

            def _epilogue01(b, blk_ps, hT_next):
                nv = min(P, SH - b * P)
                zi = ep.tile([P, 4], f32, tag="zi")
                nc.vector.tensor_scalar(out=zi[:], in0=blk_ps[:, 128:132],
                                        scalar1=1e-30, scalar2=None, op0=OP.add)
                nc.vector.reciprocal(out=zi[:], in_=zi[:])
                hdiv = ep.tile([P, P], f32, tag="hdiv")
                nc.vector.tensor_tensor(
                    out=hdiv[:].rearrange("p (h c) -> p h c", c=32),
                    in0=blk_ps[:, 0:128].rearrange("p (h c) -> p h c", c=32),
                    in1=zi[:].unsqueeze(-1).to_broadcast([P, 4, 32]),
                    op=OP.mult)
                hre = ep.tile([P, P], f32, tag="hre")
                nc.scalar.activation(out=hre[:], in_=hdiv[:], func=AF.Relu)
                trp = pp.tile([P, P], f32, tag="pp")
                nc.tensor.transpose(out=trp[:], in_=hre[:], identity=ident[:])
                nc.vector.tensor_copy(out=hT_next[:, b * P:b * P + nv],
                                      in_=trp[:, :nv])

            def _epilogue2(b, psA, psB, acc_max):
                nv = min(P, SH - b * P)
                zi = ep.tile([P, 4], f32, tag="zi")
                nc.vector.tensor_scalar(out=zi[:], in0=psB[:, 256:260],
                                        scalar1=1e-30, scalar2=None, op0=OP.add)
                nc.vector.reciprocal(out=zi[:], in_=zi[:])
                agg = ep.tile([P, 512], bf16, tag="agg")
                nc.vector.tensor_copy(out=agg[:, 0:256], in_=psA[:, 0:256])
                nc.vector.tensor_copy(out=agg[:, 256:512], in_=psB[:, 0:256])
                o2 = pp.tile([P, 512], f32, tag="pp")
                for h in range(4):
                    trp = pp.tile([P, P], bf16, tag="pp")
                    nc.tensor.transpose(out=trp[:], in_=agg[:, h * P:(h + 1) * P],
                                        identity=identbf[:])
                    aggT = ep.tile([P, P], bf16, tag="aggT")
                    nc.vector.tensor_copy(out=aggT[:], in_=trp[:])
                    nc.tensor.matmul(out=o2[:, h * P:(h + 1) * P], lhsT=aggT[:],
                                     rhs=W2bf_sb[:, h * P:(h + 1) * P],
                                     start=True, stop=True)
                o2r = ep.tile([P, 512], f32, tag="o2r")
                for h in range(4):
                    nc.scalar.activation(out=o2r[:, h * P:(h + 1) * P],
                                         in_=o2[:, h * P:(h + 1) * P],
                                         func=AF.Relu, scale=zi[:, h:h + 1])
                nc.vector.tensor_tensor(out=acc_max[:nv, :], in0=acc_max[:nv, :],
                                        in1=o2r[:nv, :], op=OP.max)

            def layer_main(l, hT_next, acc_max):
                elem = 256 if l == 2 else P
                aw = 512 if l == 2 else 128  # alpha col offset within 516-wide row
                ch0 = 0
                cur_b = 0
                done_in_b = 0
                blk_ps = blk_psB = None
                for g, blks in enumerate(groups):
                    gch = sched["grp_ch"][g]
                    o8 = int(grp_choff[g]) * 8
                    n_idx = gch * P
                    fg = gp.tile([P, MAXG, elem], bf16, tag="fg")
                    nc.gpsimd.dma_gather(
                        out_ap=fg[:, :gch, :], in_ap=ftab[l][:],
                        idxs_ap=fidx_sb[:, o8:o8 + gch * 8],
                        num_idxs=n_idx, num_idxs_reg=n_idx, elem_size=elem,
                        single_packet=False, queue_num=(g % 2) * 2)
                    eg = gp.tile([P, MAXG, P], bf16, tag="eg")
                    nc.gpsimd.dma_gather(
                        out_ap=eg[:, :gch, :], in_ap=ertab[l][:],
                        idxs_ap=eidx_sb[:, o8:o8 + gch * 8],
                        num_idxs=n_idx, num_idxs_reg=n_idx, elem_size=P,
                        single_packet=False, queue_num=(g % 2) * 2 + 1)
                    for s0 in range(0, gch, 4):
                        sub = min(4, gch - s0)
                        ch = ch0 + s0
                        gt = wk.tile([P, 4 * 516], bf16, tag="gt")
                        gtv = gt[:].rearrange("p (s w) -> p s w", w=516)
                        oh = wk.tile([P, 4, P], bf16, tag="oh")
                        nc.vector.tensor_tensor(
                            out=oh[:, :sub, :],
                            in0=dstm_sb[:, ch:ch + sub].unsqueeze(-1).to_broadcast([P, sub, P]),
                            in1=iota_sb[:].unsqueeze(1).to_broadcast([P, sub, P]),
                            op=OP.is_equal)
                        e4 = wk.tile([P, 16], f32, tag="e4")
                        e4v = e4[:, :sub * 4].rearrange("p (s h) -> p s h", h=4)
                        if l < 2:
                            tmp = wk.tile([P, 4 * P], bf16, tag="tmp")
                            nc.vector.tensor_tensor(
                                out=tmp[:, :sub * P].rearrange("p (s f) -> p s f", f=P),
                                in0=fg[:, s0:s0 + sub, :],
                                in1=alrep_sb[l][:].unsqueeze(1).to_broadcast([P, sub, P]),
                                op=OP.mult)
                            el4 = wk.tile([P, 16], f32, tag="el4")
                            nc.vector.reduce_sum(
                                out=el4[:, :sub * 4],
                                in_=tmp[:, :sub * P].rearrange("p (q c) -> p q c", c=32),
                                axis=mybir.AxisListType.X)
                            nc.vector.tensor_tensor(
                                out=e4v,
                                in0=el4[:, :sub * 4].rearrange("p (s h) -> p s h", h=4),
                                in1=eg[:, s0:s0 + sub, 0:4],
                                op=OP.add)
                        else:
                            nc.vector.tensor_tensor(
                                out=e4v,
                                in0=fg[:, s0:s0 + sub, 128:132],
                                in1=eg[:, s0:s0 + sub, 0:4],
                                op=OP.add)
                        a1 = wk.tile([P, 16], bf16, tag="a1")
                        a2 = wk.tile([P, 16], bf16, tag="a2")
                        nc.scalar.activation(out=a1[:, :sub * 4], in_=e4[:, :sub * 4],
                                             func=AF.Exp)
                        nc.scalar.activation(out=a2[:, :sub * 4], in_=e4[:, :sub * 4],
                                             func=AF.Exp, scale=0.2)
                        nc.vector.tensor_tensor(
                            out=gtv[:, :sub, aw:aw + 4],
                            in0=a1[:, :sub * 4].rearrange("p (s h) -> p s h", h=4),
                            in1=a2[:, :sub * 4].rearrange("p (s h) -> p s h", h=4),
                            op=OP.max)
                        if l < 2:
                            nc.vector.tensor_tensor(
                                out=gtv[:, :sub, 0:P].rearrange("p s (h c) -> p s h c", c=32),
                                in0=fg[:, s0:s0 + sub, :].rearrange("p s (h c) -> p s h c", c=32),
                                in1=gtv[:, :sub, aw:aw + 4].unsqueeze(-1).to_broadcast([P, sub, 4, 32]),
                                op=OP.mult)
                        else:
                            for c in range(sub):
                                nc.vector.tensor_tensor(
                                    out=gtv[:, c, 0:512].rearrange("p (h f) -> p h f", f=P),
                                    in0=fg[:, s0 + c, 0:P].unsqueeze(1).to_broadcast([P, 4, P]),
                                    in1=gtv[:, c, aw:aw + 4].unsqueeze(-1).to_broadcast([P, 4, P]),
                                    op=OP.mult)
                        for c in range(sub):
                            if done_in_b == 0:
                                blk_ps = pp.tile([P, 512], f32, tag="pp")
                                if l == 2:
                                    blk_psB = pp.tile([P, 512], f32, tag="pp")
                            first = done_in_b == 0
                            last = done_in_b == chunks_b[cur_b] - 1
                            if l < 2:
                                nc.tensor.matmul(
                                    out=blk_ps[:, 0:132], lhsT=oh[:, c, :],
                                    rhs=gt[:, c * 516:c * 516 + 132],
                                    start=first, stop=last)
                            else:
                                nc.tensor.matmul(
                                    out=blk_ps[:, 0:256], lhsT=oh[:, c, :],
                                    rhs=gt[:, c * 516:c * 516 + 256],
                                    start=first, stop=last)
                                nc.tensor.matmul(
                                    out=blk_psB[:, 0:260], lhsT=oh[:, c, :],
                                    rhs=gt[:, c * 516 + 256:c * 516 + 516],
                                    start=first, stop=last)
                            done_in_b += 1
                            if last:
                                if l < 2:
                                    _epilogue01(cur_b, blk_ps, hT_next)
                                else:
                                    _epilogue2(cur_b, blk_ps, blk_psB, acc_max)
                                done_in_b = 0
                                cur_b += 1
                    ch0 += gch

            # ================= run the network =================
            def dump_dbg(ap_f32_cols):
                """ap_f32_cols: list of (sbuf_ap, col0, width)"""
                dt_ = pers.tile([P, 512], f32, tag="dbgt")
                nc.gpsimd.memset(dt_[:], 0.0)
                for ap, c0, w in ap_f32_cols:
                    nc.vector.tensor_copy(out=dt_[:, c0:c0 + w], in_=ap)
                nc.sync.dma_start(out=dbg_ext[:], in_=dt_[:])
                dd = ep.tile([1, 8], f32, tag="ot")
                nc.gpsimd.memset(dd[:], 0.5)
                nc.sync.dma_start(out=out_ext[:], in_=dd[:])

            def dump_tab(l):
                w = 256 if l == 2 else P
                tb = ep.tile([P, w], bf16, tag="dump1")
                nc.sync.dma_start(out=tb[:], in_=ftab[l][0:P, :])
                er_ = ep.tile([P, 4], bf16, tag="dump2")
                nc.sync.dma_start(out=er_[:], in_=ertab[l][0:P, 0:4])
                cols = [(tb[:, :P], 0, P), (er_[:], 128, 4)]
                if l == 2:
                    cols.append((tb[:, 128:132], 140, 4))
                dump_dbg(cols)

            hT0 = pers.tile([P, SH], f32, tag="hT0")
            nc.sync.dma_start(out=hT0[:FIN, :], in_=xT_in[:])
            stage_prep(0, hT0)
            if phase == 0:
                dump_tab(0)
            if phase >= 1:
                hT1 = pers.tile([P, SH], f32, tag="hT1")
                layer_main(0, hT1, None)
                if phase == 1:
                    dump_dbg([(hT1[:, 0:min(512, SH)], 0, min(512, SH))])
            if phase >= 2:
                stage_prep(1, hT1)
                if phase == 2:
                    dump_tab(1)
            if phase >= 3:
                hT2 = pers.tile([P, SH], f32, tag="hT2")
                layer_main(1, hT2, None)
                if phase == 3:
                    dump_dbg([(hT2[:, 0:min(512, SH)], 0, min(512, SH))])
            if phase >= 4:
                stage_prep(2, hT2)
                if phase == 4:
                    dump_tab(2)
            if phase >= 5:
                acc_max = pers.tile([P, 512], f32, tag="accmax")
                nc.gpsimd.memset(acc_max[:], 0.0)
                layer_main(2, None, acc_max)
                if phase == 5:
                    dump_dbg([(acc_max[:], 0, 512)])
            def head():

                pooledT = ep.tile([P, 4], f32, tag="pooledT")
                for j in range(4):
                    trp = pp.tile([P, P], f32, tag="pp")
                    nc.tensor.transpose(out=trp[:], in_=acc_max[:, j * P:(j + 1) * P],
                                        identity=ident[:])
                    nc.vector.reduce_max(out=pooledT[:, j:j + 1], in_=trp[:],
                                         axis=mybir.AxisListType.X)
                nc.sync.dma_start(out=pmax_in[:], in_=pooledT[:])
                nc.gpsimd.collective_compute(
                    "AllReduce", OP.max,
                    ins=[pmax_in[:]], outs=[pmax_out[:]], replica_groups=rg)
                pm = ep.tile([P, 4], f32, tag="pm")
                nc.sync.dma_start(out=pm[:], in_=pmax_out[:])
                fcp = pp.tile([1, 8], f32, tag="pp")
                for j in range(4):
                    nc.tensor.matmul(out=fcp[:], lhsT=pm[:, j:j + 1],
                                     rhs=fcw_sb[:, j * 8:(j + 1) * 8],
                                     start=(j == 0), stop=(j == 3))
                lg = ep.tile([1, 8], f32, tag="lg")
                nc.vector.tensor_tensor(out=lg[:], in0=fcp[:], in1=fcb_sb[:], op=OP.add)
                mx = ep.tile([1, 1], f32, tag="mx")
                nc.vector.reduce_max(out=mx[:], in_=lg[:], axis=mybir.AxisListType.X)
                nc.vector.tensor_tensor(out=lg[:], in0=lg[:],
                                        in1=mx[:].to_broadcast([1, 8]), op=OP.subtract)
                ex = ep.tile([1, 8], f32, tag="ex")
                nc.scalar.activation(out=ex[:], in_=lg[:], func=AF.Exp)
                sm = ep.tile([1, 1], f32, tag="sm")
                nc.vector.reduce_sum(out=sm[:], in_=ex[:], axis=mybir.AxisListType.X)
                nc.vector.reciprocal(out=sm[:], in_=sm[:])
                ot = ep.tile([1, 8], f32, tag="ot")
                nc.vector.tensor_tensor(out=ot[:], in0=ex[:],
                                        in1=sm[:].to_broadcast([1, 8]), op=OP.mult)
                nc.sync.dma_start(out=out_ext[:], in_=ot[:])

            # ---- head ----
            if phase >= 6:
                head()

    nc.finalize()
    return nc


def _host_consts(W0, al0, ar0, W1, al1, ar1, W2, al2, ar2, fc_w, fc_b):
    def alflat(al):
        v = al.reshape(-1).astype(np.float32)
        return np.tile(v[None, :], (P, 1)).astype(BF16)

    def armat(ar):
        hh, cc = ar.shape
        m = np.zeros((hh * cc, hh), np.float32)
        for h in range(hh):
            m[h * cc:(h + 1) * cc, h] = ar[h]
        return m

    wal2 = np.zeros((P, 8), np.float32)
    wal2[:, 0:4] = (W2.astype(np.float64) @ armat(al2).astype(np.float64)).astype(np.float32)
    wal2[:, 4:8] = (W2.astype(np.float64) @ armat(ar2).astype(np.float64)).astype(np.float32)
    fcw = np.ascontiguousarray(
        fc_w.reshape(4, P, 8).transpose(1, 0, 2).reshape(P, 32)).astype(np.float32)
    return {
        "W0": np.ascontiguousarray(W0).astype(np.float32),
        "W1": np.ascontiguousarray(W1).astype(np.float32),
        "W2bf": np.ascontiguousarray(W2).astype(BF16),
        "alrep0": alflat(al0), "alrep1": alflat(al1),
        "armat0": armat(ar0), "armat1": armat(ar1),
        "wal2": wal2,
        "fcw": fcw, "fcb": fc_b.reshape(1, 8).astype(np.float32),
        "iota": np.tile(np.arange(P, dtype=np.float32)[None, :], (P, 1)).astype(BF16),
    }


_PROG_CACHE = {}


def run_gat(inputs, src, dst, W0, al0, ar0, W1, al1, ar1, W2, al2, ar2, fc_w, fc_b,
            trace=False):
    from concourse.bass_utils import run_bass_kernel_spmd
    inputs = np.asarray(inputs)
    N, FIN = inputs.shape
    E = np.asarray(src).shape[0]
    sched, core_arrays = _preprocess(np.asarray(src), np.asarray(dst), N, E)
    import os
    phase = int(os.environ.get("GAT_PHASE", "6"))
    key = (N, E, FIN, tuple(sched["chunks_b"]), phase)
    if key not in _PROG_CACHE:
        _PROG_CACHE[key] = _build_program(sched, FIN, phase)
    nc = _PROG_CACHE[key]
    consts = _host_consts(np.asarray(W0), np.asarray(al0), np.asarray(ar0),
                          np.asarray(W1), np.asarray(al1), np.asarray(ar1),
                          np.asarray(W2), np.asarray(al2), np.asarray(ar2),
                          np.asarray(fc_w), np.asarray(fc_b))
    SH = sched["SH"]
    in_maps = []
    for c in range(NC):
        m = dict(consts)
        m.update(core_arrays[c])
        m["xT"] = np.ascontiguousarray(
            inputs[c * SH:(c + 1) * SH, :].T).astype(np.float32)
        in_maps.append(m)
    res = run_bass_kernel_spmd(nc, in_maps, list(range(NC)), trace=trace)
    out = np.asarray(res.results[0]["out"])
    run_gat.last_dbg = np.asarray(res.results[0].get("dbg")) if "dbg" in res.results[0] else None
    return out, res


def kernel(**inputs):
    out, _ = run_gat(**inputs)
    return out

